# revision 1
# baseline (speedup 1.0000x reference)
"""Trainium2 Bass kernel for nn_DLI_loss_2 (ragged_sequence).

Reference computation (B=16, S=4096, E=1024, T=32, H=512):
    states[b,k,:] = encoder_output[b, ids[b,k], :]          (ragged gather)
    ... 2-step LSTM -> h2 -> a = h2 @ w_h + fc_b            (per (b,j) scalar)
    t = states @ w_t                                        (w_t = fc_w[0, H:])
    logits[b,j,k] = a[b,j] + t[b,k]  masked to k >= j+2
    loss = mean_j( logsumexp_k logits - (a[b,j] + t[b,j+2]) )

Since a[b,j] is constant over k, logsumexp_k(a+t) = a + logsumexp_k(t), so the
a term (the whole LSTM + fc_b path) cancels exactly:
    loss = mean_{b,j}[ log(sum_{k>=j+2} exp(t[b,k])) - t[b,j+2] ]

exp() is safe without max-subtraction here: |t| <= ~6 for any plausible input
scale (t is a 1024-dim dot with weights ~U(+-1/sqrt(1536))), and the result
matches the reference's max-subtracted logsumexp to fp32 rounding.

Per-core program (data-parallel over batch, 2 batches/core on 8 cores):
    1. indirect-DMA gather of the 64 turn-end rows from HBM (gated only by a
       tiny index DMA; the weight/mask DMA runs in parallel on the ACT HWDGE)
    2. DVE mul + reduce: t[64,1] = sum_E(states * w_t)
    3. ACT exp -> e[64,1]   (exp/ln/relu tables pre-warmed during the DMAs)
    4. PE matmuls vs a constant 0/1 suffix mask: S[1,60] and C[1,1]
    5. ACT ln with fused row-sum -> red = sum_j log S
       ACT relu(red - C) -> per-core partial (each term is >= 0 by
       construction: logsumexp over a set containing the correct logit)
Host sums the 8 per-core partials and divides by B*(T-2).

Raw bass with explicit semaphores: every instruction carries at most ONE
sync-wait (this toolchain's walrus rejects multi-wait instruction structs);
extra dependencies are covered transitively through the wait chain: exp waits
s_dve>=2 which implies the gather and both input DMAs completed, so the PE
matmuls (which read the mask) only need s_act, and the final relu only needs
s_pe via ln's wait.
"""

import numpy as np

B, S, E, T, H = 16, 4096, 1024, 32, 512
NCORES = 8
B_LOC = B // NCORES          # batches per core
R = B_LOC * T                # gathered rows per core (64)
NJ = B_LOC * (T - 2)         # loss terms per core (60)
DCOLS = E + NJ + 2           # data tensor: w_t | mask | v | zeros(bias)

_CACHE = {}


def _build_nc():
    from contextlib import ExitStack

    import concourse.bass as bass
    import concourse.mybir as mybir

    f32 = mybir.dt.float32
    i32 = mybir.dt.int32
    AF = mybir.ActivationFunctionType

    nc = bass.Bass("TRN2", target_bir_lowering=False, debug=False)
    # trim the init preamble: the 4 const-tile memsets and the all-engine
    # barrier are dead weight here (biases come from dat_sb, and our own
    # semaphores express every cross-engine dependency)
    _root = nc.m.functions[0].blocks[0]
    _keep = [
        i
        for i in _root.instructions
        if not (
            type(i).__name__ in ("InstMemset", "InstDrain")
            or i.name.startswith("barrier_")
        )
    ]
    del _root.instructions[:]
    _root.instructions.extend(_keep)

    enc = nc.dram_tensor("enc", [B_LOC * S, E], f32, kind="ExternalInput").ap()
    gidx = nc.dram_tensor("gidx", [R, 1], i32, kind="ExternalInput").ap()
    data = nc.dram_tensor("data", [R, DCOLS], f32, kind="ExternalInput").ap()
    out = nc.dram_tensor("out", [1, 1], f32, kind="ExternalOutput").ap()

    with ExitStack() as ctx:
        blk = ctx.enter_context(nc.Block())
        s_idx = ctx.enter_context(nc.semaphore("s_idx"))
        s_in = ctx.enter_context(nc.semaphore("s_in"))
        s_g = ctx.enter_context(nc.semaphore("s_g"))
        s_w = ctx.enter_context(nc.semaphore("s_w"))
        s_dve = ctx.enter_context(nc.semaphore("s_dve"))
        s_act = ctx.enter_context(nc.semaphore("s_act"))
        s_pe = ctx.enter_context(nc.semaphore("s_pe"))
        s_out = ctx.enter_context(nc.semaphore("s_out"))

        idx_sb = ctx.enter_context(nc.sbuf_tensor("idx_sb", [R, 1], i32))
        dat_sb = ctx.enter_context(nc.sbuf_tensor("dat_sb", [R, DCOLS], f32))
        st_sb = ctx.enter_context(nc.sbuf_tensor("st_sb", [R, E], f32))
        prod = ctx.enter_context(nc.sbuf_tensor("prod", [R, E], f32))
        t_sb = ctx.enter_context(nc.sbuf_tensor("t_sb", [R, 1], f32))
        e_sb = ctx.enter_context(nc.sbuf_tensor("e_sb", [R, 1], f32))
        warm_sb = ctx.enter_context(nc.sbuf_tensor("warm_sb", [1, 5], f32))
        lse_sb = ctx.enter_context(nc.sbuf_tensor("lse_sb", [1, NJ], f32))
        red_sb = ctx.enter_context(nc.sbuf_tensor("red_sb", [1, 1], f32))
        res_sb = ctx.enter_context(nc.sbuf_tensor("res_sb", [1, 1], f32))
        s_ps = ctx.enter_context(nc.psum_tensor("s_ps", [1, NJ], f32))
        c_ps = ctx.enter_context(nc.psum_tensor("c_ps", [1, 1], f32))

        wt_v = dat_sb[:, :E]                 # [64, 1024] w_t replicated
        mskS_v = dat_sb[:, E : E + NJ]       # [64, 60] suffix mask
        mskC_v = dat_sb[:, E + NJ : E + NJ + 1]  # [64, 1] correct-term mask
        zb64_v = dat_sb[:, E + NJ + 1 : E + NJ + 2]  # [64, 1] zeros (act bias)
        zb1_v = dat_sb[:1, E + NJ + 1 : E + NJ + 2]  # [1, 1] zeros (act bias)

        @blk.sync
        def _(sync):
            sync.dma_start(idx_sb[:], gidx).then_inc(s_idx, 16)
            sync.wait_ge(s_act, 3)
            sync.dma_start(out, res_sb[:]).then_inc(s_out, 16)

        @blk.gpsimd
        def _(gpsimd):
            gpsimd.wait_ge(s_idx, 16)
            gpsimd.indirect_dma_start(
                out=st_sb[:],
                out_offset=None,
                in_=enc,
                in_offset=bass.IndirectOffsetOnAxis(ap=idx_sb[:, :1], axis=0),
            ).then_inc(s_g, 16)

        @blk.scalar
        def _(scalar):
            scalar.dma_start(dat_sb[:], data).then_inc(s_in, 16)
            # warm the activation tables while the DMAs are in flight
            scalar.wait_ge(s_w, 1)
            scalar.activation(
                out=warm_sb[:, 1:2], in_=warm_sb[:, :1], func=AF.Exp,
                bias=warm_sb[:1, :1],
            )
            scalar.activation(
                out=warm_sb[:, 2:3], in_=warm_sb[:, :1], func=AF.Ln,
                bias=warm_sb[:1, :1],
            )
            scalar.activation(
                out=warm_sb[:, 3:4], in_=warm_sb[:, :1], func=AF.Relu,
                bias=warm_sb[:1, :1],
            )
            scalar.wait_ge(s_dve, 2)
            scalar.activation(
                out=e_sb[:], in_=t_sb[:], func=AF.Exp, bias=zb64_v
            ).then_inc(s_act, 1)
            scalar.wait_ge(s_pe, 2)
            scalar.activation(
                out=lse_sb[:],
                in_=s_ps[:],
                func=AF.Ln,
                bias=zb1_v,
                accum_out=red_sb[:],
            ).then_inc(s_act, 1)
            scalar.wait_ge(s_act, 2)
            # res = relu(red - C); every loss term is >= 0 so relu is exact
            scalar.activation(
                out=res_sb[:],
                in_=c_ps[:],
                func=AF.Relu,
                bias=red_sb[:1, :1],
                scale=-1.0,
            ).then_inc(s_act, 1)

        @blk.vector
        def _(vector):
            vector.memset(warm_sb[:], 1.0).then_inc(s_w, 1)
            vector.wait_ge(s_in, 16)
            vector.wait_ge(s_g, 16)
            vector.tensor_mul(out=prod[:], in0=st_sb[:], in1=wt_v).then_inc(
                s_dve, 1
            )
            vector.wait_ge(s_dve, 1)
            vector.tensor_reduce(
                out=t_sb[:],
                in_=prod[:],
                axis=mybir.AxisListType.X,
                op=mybir.AluOpType.add,
            ).then_inc(s_dve, 1)

        @blk.tensor
        def _(tensor):
            # C = v . t can run during exp; S = msk . e follows it
            tensor.wait_ge(s_dve, 2)
            tensor.matmul(
                out=c_ps[:], lhsT=t_sb[:, :1], rhs=mskC_v, start=True, stop=True
            ).then_inc(s_pe, 1)
            tensor.wait_ge(s_act, 1)
            tensor.matmul(
                out=s_ps[:], lhsT=e_sb[:, :1], rhs=mskS_v, start=True, stop=True
            ).then_inc(s_pe, 1)

    # trim the end-of-program all-engine barrier (drain + EVSEM butterfly):
    # the SP program already ends with an explicit wait for the output DMA,
    # so engines can halt independently
    for _b in nc.m.functions[0].blocks:
        if _b.name.endswith("_end"):
            _tail_keep = [
                i
                for i in _b.instructions
                if not (
                    type(i).__name__ == "InstDrain" or i.name.startswith("barrier_")
                )
            ]
            del _b.instructions[:]
            _b.instructions.extend(_tail_keep)
    return nc


def _get_nc():
    if "nc" not in _CACHE:
        _CACHE["nc"] = _build_nc()
    return _CACHE["nc"]


def _build_mask():
    # cols 0..NJ-1: msk[b*T+k, b2*(T-2)+j] = (b==b2) and (k >= j+2)
    # col NJ: 1 if k >= 2 (selects the "correct" logits for the t-sum)
    m = np.zeros((R, NJ + 1), dtype=np.float32)
    for b in range(B_LOC):
        for k in range(T):
            for j in range(T - 2):
                if k >= j + 2:
                    m[b * T + k, b * (T - 2) + j] = 1.0
            if k >= 2:
                m[b * T + k, NJ] = 1.0
    return m


def kernel(encoder_output, his_turn_end_ids, w_ih, w_hh, b_ih, b_hh, fc_w, fc_b):
    from concourse import bass_utils

    nc = _get_nc()
    enc = np.ascontiguousarray(np.asarray(encoder_output, dtype=np.float32))
    ids = np.asarray(his_turn_end_ids)
    w_t = np.asarray(fc_w, dtype=np.float32)[0, H:]  # [E]

    data = np.zeros((R, DCOLS), dtype=np.float32)
    data[:, :E] = w_t[None, :]
    data[:, E : E + NJ + 1] = _build_mask()
    data = np.ascontiguousarray(data)

    in_maps = []
    for c in range(NCORES):
        b0 = c * B_LOC
        enc_l = enc[b0 : b0 + B_LOC].reshape(B_LOC * S, E)
        gidx = (
            ids[b0 : b0 + B_LOC].astype(np.int64)
            + (np.arange(B_LOC, dtype=np.int64) * S)[:, None]
        ).reshape(R, 1).astype(np.int32)
        in_maps.append({"enc": enc_l, "gidx": gidx, "data": data})

    try:
        res = bass_utils.run_bass_kernel_spmd(
            nc, in_maps, core_ids=list(range(NCORES))
        )
    except ModuleNotFoundError:
        # ambient BASS_TRACE with no NTFF hook module on this image --
        # rerun with tracing hard-disabled
        import os

        os.environ["BASS_NEVER_TRACE"] = "1"
        res = bass_utils.run_bass_kernel_spmd(
            nc, in_maps, core_ids=list(range(NCORES))
        )
    _CACHE["last_results"] = res
    total = sum(float(r["out"][0, 0]) for r in res.results)
    return np.float32(total / (B * (T - 2)))



# revision 12
# speedup vs baseline: 1.0160x; 1.0160x over previous
"""Trainium2 Bass kernel for nn_DLI_loss_2 (ragged_sequence).

Reference computation (B=16, S=4096, E=1024, T=32, H=512):
    states[b,k,:] = encoder_output[b, ids[b,k], :]          (ragged gather)
    ... 2-step LSTM -> h2 -> a = h2 @ w_h + fc_b            (per (b,j) scalar)
    t = states @ w_t                                        (w_t = fc_w[0, H:])
    logits[b,j,k] = a[b,j] + t[b,k]  masked to k >= j+2
    loss = mean_j( logsumexp_k logits - (a[b,j] + t[b,j+2]) )

Since a[b,j] is constant over k, logsumexp_k(a+t) = a + logsumexp_k(t), so the
a term (the whole LSTM + fc_b path) cancels exactly:
    loss = mean_{b,j}[ log(sum_{k>=j+2} exp(t[b,k])) - t[b,j+2] ]

exp() is safe without max-subtraction here: |t| <= ~6 for any plausible input
scale (t is a 1024-dim dot with weights ~U(+-1/sqrt(1536))).

Per-core program (data-parallel over batch, 2 batches/core on 8 cores).
Every DMA here costs ~2.2-3.3us END TO END in fixed overhead (HWDGE/SWDGE
descriptor generation + DGE start delay + completion-semaphore propagation);
transfer time is negligible at these sizes. The layout below minimizes the
DMA chain on the critical path:

  1. "mini" DMA (critical path, ACT queue, which has the earliest program
     prologue of the DMA-capable engines): one [128 x 63] tensor carrying
     the gather offsets (int32 bit-cast into f32 storage), the 0/1 suffix
     masks, and zeros used as activation biases. Tiny rows -> fast transfer.
  2. "big" DMA (SP queue, fully hidden behind 1): w_t split into halves
     across 128 partitions + the 128x64 pair-sum matrix for the PE.
  3. Gather: the 64 turn-end rows are fetched as 128 HALF-rows via one pair
     of indirect DMAs over enc viewed as [B_LOC*S*2, 512]: partition v<64
     holds the first 2KB of row v, partition v+64 the second 2KB. This
     fills all 128 DVE lanes (vs 64 in the row-per-partition layout) and
     lets the first half-gather overlap the second.
  4. DVE tensor_tensor_reduce (fused mul+row-sum, one pass per half):
     red[v] = dot(half-row v, w_t half) -> [128,1]
  5. PE: t = pairM^T @ red recombines halves -> t[64,1] in PSUM;
     C = red . mskC2 (the sum of "correct" logits) runs right after.
  6. ACT exp(t) -> e[64,1]; PE: S[1,60] = e^T @ suffix-mask;
     ACT ln(S) with fused row-sum -> red_s = sum_j log S_j;
     ACT relu(red_s - C) -> per-core partial (each term >= 0 by
     construction); out-DMA from the ACT queue.
Host sums the 8 per-core partials and divides by B*(T-2).

Raw bass with explicit semaphores; every instruction carries at most ONE
sync-wait (walrus rejects multi-wait instruction structs); extra deps are
covered transitively through the wait chain.
"""

import numpy as np

B, S, E, T, H = 16, 4096, 1024, 32, 512
NCORES = 8
B_LOC = B // NCORES          # batches per core
R = B_LOC * T                # gathered rows per core (64)
NJ = B_LOC * (T - 2)         # loss terms per core (60)
EH = E // 2                  # half-row length (512)
V = 2 * R                    # half-rows = partitions used (128)
# mini tensor columns: offsets | suffix mask | correct mask | zeros
MCOLS = 1 + NJ + 1 + 1       # 63
BCOLS = EH + R               # big tensor: w_t halves | pair matrix (576)

_CACHE = {}


def _build_nc():
    from contextlib import ExitStack

    import concourse.bass as bass
    import concourse.mybir as mybir

    f32 = mybir.dt.float32
    i32 = mybir.dt.int32
    AF = mybir.ActivationFunctionType

    nc = bass.Bass("TRN2", target_bir_lowering=False, debug=False)
    # trim the init preamble: the 4 const-tile memsets and the all-engine
    # barrier are dead weight here (biases come from mini_sb, and our own
    # semaphores express every cross-engine dependency)
    _root = nc.m.functions[0].blocks[0]
    _keep = [
        i
        for i in _root.instructions
        if not (
            type(i).__name__ in ("InstMemset", "InstDrain")
            or i.name.startswith("barrier_")
        )
    ]
    del _root.instructions[:]
    _root.instructions.extend(_keep)

    # enc viewed as half-rows: full row r -> half-rows 2r (first 2KB), 2r+1
    enc = nc.dram_tensor("enc", [B_LOC * S * 2, EH], f32, kind="ExternalInput").ap()
    mini = nc.dram_tensor("mini", [V, MCOLS], f32, kind="ExternalInput").ap()
    big = nc.dram_tensor("big", [V, BCOLS], f32, kind="ExternalInput").ap()
    out = nc.dram_tensor("out", [1, 1], f32, kind="ExternalOutput").ap()

    with ExitStack() as ctx:
        blk = ctx.enter_context(nc.Block())
        s_mini = ctx.enter_context(nc.semaphore("s_mini"))
        s_big = ctx.enter_context(nc.semaphore("s_big"))
        s_g1 = ctx.enter_context(nc.semaphore("s_g1"))
        s_g2 = ctx.enter_context(nc.semaphore("s_g2"))
        s_w = ctx.enter_context(nc.semaphore("s_w"))
        s_dve = ctx.enter_context(nc.semaphore("s_dve"))
        s_act = ctx.enter_context(nc.semaphore("s_act"))
        s_pe = ctx.enter_context(nc.semaphore("s_pe"))
        s_out = ctx.enter_context(nc.semaphore("s_out"))

        mini_sb = ctx.enter_context(nc.sbuf_tensor("mini_sb", [V, MCOLS], f32))
        big_sb = ctx.enter_context(nc.sbuf_tensor("big_sb", [V, BCOLS], f32))
        st_sb = ctx.enter_context(nc.sbuf_tensor("st_sb", [V, EH], f32))
        prod = ctx.enter_context(nc.sbuf_tensor("prod", [V, EH], f32))
        red_sb = ctx.enter_context(nc.sbuf_tensor("red_sb", [V, 1], f32))
        e_sb = ctx.enter_context(nc.sbuf_tensor("e_sb", [R, 1], f32))
        warm_sb = ctx.enter_context(nc.sbuf_tensor("warm_sb", [1, 5], f32))
        lse_sb = ctx.enter_context(nc.sbuf_tensor("lse_sb", [1, NJ], f32))
        rs_sb = ctx.enter_context(nc.sbuf_tensor("rs_sb", [1, 1], f32))
        res_sb = ctx.enter_context(nc.sbuf_tensor("res_sb", [1, 1], f32))
        t_ps = ctx.enter_context(nc.psum_tensor("t_ps", [R, 1], f32))
        c_ps = ctx.enter_context(nc.psum_tensor("c_ps", [1, 1], f32))
        s_ps = ctx.enter_context(nc.psum_tensor("s_ps", [1, NJ], f32))

        offs_v = mini_sb[:, :1].bitcast(i32)         # [128,1] gather offsets
        mskS_v = mini_sb[:R, 1 : 1 + NJ]             # [64,60] suffix mask
        mskC_v = mini_sb[:, 1 + NJ : 2 + NJ]         # [128,1] correct mask
        zb64_v = mini_sb[:R, 2 + NJ : 3 + NJ]        # [64,1] zeros (act bias)
        zb1_v = mini_sb[:1, 2 + NJ : 3 + NJ]         # [1,1] zeros (act bias)
        wt_v = big_sb[:, :EH]                        # [128,512] w_t halves
        pair_v = big_sb[:, EH : EH + R]              # [128,64] pair-sum matrix

        @blk.sync
        def _(sync):
            sync.dma_start(big_sb[:], big).then_inc(s_big, 16)

        @blk.vector
        def _(vector):
            vector.memset(warm_sb[:], 1.0).then_inc(s_w, 1)
            vector.wait_ge(s_big, 16)
            vector.wait_ge(s_g1, 16)
            vector.tensor_mul(
                out=prod[:R, :], in0=st_sb[:R, :], in1=wt_v[:R, :]
            ).then_inc(s_dve, 1)
            vector.wait_ge(s_dve, 1)
            vector.tensor_reduce(
                out=red_sb[:R, :],
                in_=prod[:R, :],
                axis=mybir.AxisListType.X,
                op=mybir.AluOpType.add,
            ).then_inc(s_dve, 1)
            vector.tensor_mul(
                out=prod[R:, :], in0=st_sb[R:, :], in1=wt_v[R:, :]
            ).then_inc(s_dve, 1)
            vector.wait_ge(s_dve, 3)
            vector.tensor_reduce(
                out=red_sb[R:, :],
                in_=prod[R:, :],
                axis=mybir.AxisListType.X,
                op=mybir.AluOpType.add,
            ).then_inc(s_dve, 1)

        @blk.gpsimd
        def _(gpsimd):
            gpsimd.wait_ge(s_mini, 16)
            gpsimd.indirect_dma_start(
                out=st_sb[:],
                out_offset=None,
                in_=enc,
                in_offset=bass.IndirectOffsetOnAxis(ap=offs_v[:], axis=0),
            ).then_inc(s_g1, 16)

        @blk.scalar
        def _(scalar):
            scalar.dma_start(mini_sb[:], mini).then_inc(s_mini, 16)
            # warm the activation tables while the DMAs are in flight
            scalar.wait_ge(s_w, 1)
            scalar.activation(
                out=warm_sb[:, 1:2], in_=warm_sb[:, :1], func=AF.Exp,
                bias=warm_sb[:1, :1],
            )
            scalar.activation(
                out=warm_sb[:, 2:3], in_=warm_sb[:, :1], func=AF.Ln,
                bias=warm_sb[:1, :1],
            )
            scalar.activation(
                out=warm_sb[:, 3:4], in_=warm_sb[:, :1], func=AF.Relu,
                bias=warm_sb[:1, :1],
            )
            scalar.wait_ge(s_pe, 1)
            scalar.activation(
                out=e_sb[:], in_=t_ps[:], func=AF.Exp, bias=zb64_v
            ).then_inc(s_act, 1)
            scalar.wait_ge(s_pe, 3)
            scalar.activation(
                out=lse_sb[:],
                in_=s_ps[:],
                func=AF.Ln,
                bias=zb1_v,
                accum_out=rs_sb[:],
            ).then_inc(s_act, 1)
            scalar.wait_ge(s_act, 2)
            # res = relu(rs - C); every loss term is >= 0 so relu is exact
            scalar.activation(
                out=res_sb[:],
                in_=c_ps[:],
                func=AF.Relu,
                bias=rs_sb[:1, :1],
                scale=-1.0,
            ).then_inc(s_act, 1)
            scalar.wait_ge(s_act, 3)
            scalar.dma_start(out, res_sb[:]).then_inc(s_out, 16)

        @blk.tensor
        def _(tensor):
            tensor.wait_ge(s_dve, 4)
            # t[64,1] = pairM^T @ red  (recombine half-row dots)
            tensor.matmul(
                out=t_ps[:], lhsT=pair_v, rhs=red_sb[:], start=True, stop=True
            ).then_inc(s_pe, 1)
            # C = red . mskC2 (correct-logit sum; halves add transparently)
            tensor.matmul(
                out=c_ps[:], lhsT=red_sb[:], rhs=mskC_v, start=True, stop=True
            ).then_inc(s_pe, 1)
            tensor.wait_ge(s_act, 1)
            tensor.matmul(
                out=s_ps[:], lhsT=e_sb[:, :1], rhs=mskS_v, start=True, stop=True
            ).then_inc(s_pe, 1)

    # trim the end-of-program all-engine barrier (drain + EVSEM butterfly):
    # engines can halt independently; the final drain flushes the out-DMA
    for _b in nc.m.functions[0].blocks:
        if _b.name.endswith("_end"):
            _tail_keep = [
                i
                for i in _b.instructions
                if not (
                    type(i).__name__ == "InstDrain" or i.name.startswith("barrier_")
                )
            ]
            del _b.instructions[:]
            _b.instructions.extend(_tail_keep)
    return nc


def _get_nc():
    if "nc" not in _CACHE:
        _CACHE["nc"] = _build_nc()
    return _CACHE["nc"]


def _build_mini_consts():
    """Constant part of the mini tensor (masks + zeros); col 0 (offsets)
    filled per core."""
    m = np.zeros((V, MCOLS), dtype=np.float32)
    # suffix mask (rows 0..63 only): msk[b*T+k, b2*(T-2)+j] = (b==b2, k>=j+2)
    for b in range(B_LOC):
        for k in range(T):
            for j in range(T - 2):
                if k >= j + 2:
                    m[b * T + k, 1 + b * (T - 2) + j] = 1.0
    # correct mask on all 128 half-rows: k >= 2 (halves sum via the matmul)
    for v in range(V):
        if (v % R) % T >= 2:
            m[v, 1 + NJ] = 1.0
    return m


def _build_big():
    """w_t goes in per core (depends on fc_w); pair matrix is constant."""
    p = np.zeros((V, BCOLS), dtype=np.float32)
    for v in range(V):
        p[v, EH + (v % R)] = 1.0
    return p


def kernel(encoder_output, his_turn_end_ids, w_ih, w_hh, b_ih, b_hh, fc_w, fc_b):
    from concourse import bass_utils

    nc = _get_nc()
    enc = np.ascontiguousarray(np.asarray(encoder_output, dtype=np.float32))
    ids = np.asarray(his_turn_end_ids)
    w_t = np.asarray(fc_w, dtype=np.float32)[0, H:]  # [E]

    mini = _build_mini_consts()
    big = _build_big()
    # w_t halves: partition v holds w_t[512*(v//64) : 512*(v//64)+512]
    big[:R, :EH] = w_t[None, :EH]
    big[R:, :EH] = w_t[None, EH:]
    big = np.ascontiguousarray(big)

    in_maps = []
    for c in range(NCORES):
        b0 = c * B_LOC
        enc_l = enc[b0 : b0 + B_LOC].reshape(B_LOC * S * 2, EH)
        gidx = (
            ids[b0 : b0 + B_LOC].astype(np.int64)
            + (np.arange(B_LOC, dtype=np.int64) * S)[:, None]
        ).reshape(R)
        # half-row offsets: v<64 -> 2*gidx[v], v>=64 -> 2*gidx[v-64]+1
        offs = np.concatenate([2 * gidx, 2 * gidx + 1]).astype(np.int32)
        m = mini.copy()
        m[:, 0] = offs.view(np.float32)
        in_maps.append({"enc": enc_l, "mini": np.ascontiguousarray(m), "big": big})

    try:
        res = bass_utils.run_bass_kernel_spmd(
            nc, in_maps, core_ids=list(range(NCORES))
        )
    except ModuleNotFoundError:
        # ambient BASS_TRACE with no NTFF hook module on this image --
        # rerun with tracing hard-disabled
        import os

        os.environ["BASS_NEVER_TRACE"] = "1"
        res = bass_utils.run_bass_kernel_spmd(
            nc, in_maps, core_ids=list(range(NCORES))
        )
    _CACHE["last_results"] = res
    total = sum(float(r["out"][0, 0]) for r in res.results)
    return np.float32(total / (B * (T - 2)))


# revision 14
# speedup vs baseline: 1.0316x; 1.0153x over previous
"""Trainium2 Bass kernel for nn_DLI_loss_2 (ragged_sequence).

Reference computation (B=16, S=4096, E=1024, T=32, H=512):
    states[b,k,:] = encoder_output[b, ids[b,k], :]          (ragged gather)
    ... 2-step LSTM -> h2 -> a = h2 @ w_h + fc_b            (per (b,j) scalar)
    t = states @ w_t                                        (w_t = fc_w[0, H:])
    logits[b,j,k] = a[b,j] + t[b,k]  masked to k >= j+2
    loss = mean_j( logsumexp_k logits - (a[b,j] + t[b,j+2]) )

Since a[b,j] is constant over k, logsumexp_k(a+t) = a + logsumexp_k(t), so the
a term (the whole LSTM + fc_b path) cancels exactly:
    loss = mean_{b,j}[ log(sum_{k>=j+2} exp(t[b,k])) - t[b,j+2] ]

exp() is safe without max-subtraction here: |t| <= ~6 for any plausible input
scale (t is a 1024-dim dot with weights ~U(+-1/sqrt(1536))).

Per-core program (data-parallel over batch, 2 batches/core on 8 cores).
Every DMA costs ~2.2-3.3us END TO END in fixed overhead (descriptor
generation + DGE start delay + completion-semaphore propagation); transfer
time is negligible at these sizes. The structure minimizes the DMA chain on
the critical path and the compute chain after it:

  1. "mini" DMA (critical path, SP queue): [128 x 63] tensor with the gather
     offsets (int32 bit-cast into f32 storage), the 0/1 suffix masks and
     zeros used as activation biases. Tiny rows -> fast transfer.
  2. ACT queue (hidden behind 1): "wtst" carries per-partition w_t halves;
     "pairm" carries the 128x64 half-recombine matrix.
  3. Gather: enc viewed as [B_LOC*S*2, 512] half-rows; one indirect DMA
     fetches 128 half-rows: partition v<64 holds the first 2KB of row v,
     partition v+64 the second 2KB. This fills all 128 DVE lanes (vs 64 in
     a row-per-partition layout).
  4. DVE mul + reduce: red[v] = dot(st[v], w_t half) -> [128,1].
  5. PE: t = pairM^T @ red -> t[64,1] in PSUM (recombines halves);
     C = red . mskC2 (sum of "correct" logits) runs right behind it.
  6. ACT exp(t) -> e[64,1]; PE: S[1,60] = e^T @ suffix-mask;
     ACT ln(S) with fused row-sum -> rs = sum_j log S_j;
     ACT relu(rs - C) -> per-core partial (each term >= 0 by construction);
     out-DMA from the SP queue.
Host sums the 8 per-core partials and divides by B*(T-2).

Raw bass with explicit semaphores; every instruction carries at most ONE
sync-wait (walrus rejects multi-wait instruction structs); extra deps are
covered transitively through the wait chain.
"""

import numpy as np

B, S, E, T, H = 16, 4096, 1024, 32, 512
NCORES = 8
B_LOC = B // NCORES          # batches per core
R = B_LOC * T                # gathered rows per core (64)
NJ = B_LOC * (T - 2)         # loss terms per core (60)
EH = E // 2                  # half-row length (512)
V = 2 * R                    # half-rows = partitions used (128)
# mini tensor columns: offsets | suffix mask | correct mask | zeros
MCOLS = 1 + NJ + 1 + 1       # 63

_CACHE = {}


def _build_nc():
    from contextlib import ExitStack

    import concourse.bass as bass
    import concourse.mybir as mybir

    f32 = mybir.dt.float32
    i32 = mybir.dt.int32
    AF = mybir.ActivationFunctionType

    nc = bass.Bass("TRN2", target_bir_lowering=False, debug=False)
    # trim the init preamble: the 4 const-tile memsets and the all-engine
    # barrier are dead weight here (biases come from mini_sb, and our own
    # semaphores express every cross-engine dependency)
    _root = nc.m.functions[0].blocks[0]
    _keep = [
        i
        for i in _root.instructions
        if not (
            type(i).__name__ in ("InstMemset", "InstDrain")
            or i.name.startswith("barrier_")
        )
    ]
    del _root.instructions[:]
    _root.instructions.extend(_keep)

    # enc viewed as half-rows: full row r -> half-rows 2r (first 2KB), 2r+1
    enc = nc.dram_tensor("enc", [B_LOC * S * 2, EH], f32, kind="ExternalInput").ap()
    mini = nc.dram_tensor("mini", [V, MCOLS], f32, kind="ExternalInput").ap()
    wtst = nc.dram_tensor("wtst", [V, EH], f32, kind="ExternalInput").ap()
    pairm = nc.dram_tensor("pairm", [V, R], f32, kind="ExternalInput").ap()
    out = nc.dram_tensor("out", [1, 1], f32, kind="ExternalOutput").ap()

    with ExitStack() as ctx:
        blk = ctx.enter_context(nc.Block())
        s_mini = ctx.enter_context(nc.semaphore("s_mini"))
        s_wt = ctx.enter_context(nc.semaphore("s_wt"))
        s_pm = ctx.enter_context(nc.semaphore("s_pm"))
        s_g = ctx.enter_context(nc.semaphore("s_g"))
        s_w = ctx.enter_context(nc.semaphore("s_w"))
        s_dve = ctx.enter_context(nc.semaphore("s_dve"))
        s_act = ctx.enter_context(nc.semaphore("s_act"))
        s_pe = ctx.enter_context(nc.semaphore("s_pe"))
        s_out = ctx.enter_context(nc.semaphore("s_out"))

        mini_sb = ctx.enter_context(nc.sbuf_tensor("mini_sb", [V, MCOLS], f32))
        pair_sb = ctx.enter_context(nc.sbuf_tensor("pair_sb", [V, R], f32))
        st_sb = ctx.enter_context(nc.sbuf_tensor("st_sb", [V, EH], f32))
        wt_sb = ctx.enter_context(nc.sbuf_tensor("wt_sb", [V, EH], f32))
        prod = ctx.enter_context(nc.sbuf_tensor("prod", [V, EH], f32))
        red_sb = ctx.enter_context(nc.sbuf_tensor("red_sb", [V, 1], f32))
        e_sb = ctx.enter_context(nc.sbuf_tensor("e_sb", [R, 1], f32))
        warm_sb = ctx.enter_context(nc.sbuf_tensor("warm_sb", [1, 5], f32))
        lse_sb = ctx.enter_context(nc.sbuf_tensor("lse_sb", [1, NJ], f32))
        rs_sb = ctx.enter_context(nc.sbuf_tensor("rs_sb", [1, 1], f32))
        res_sb = ctx.enter_context(nc.sbuf_tensor("res_sb", [1, 1], f32))
        t_ps = ctx.enter_context(nc.psum_tensor("t_ps", [R, 1], f32))
        c_ps = ctx.enter_context(nc.psum_tensor("c_ps", [1, 1], f32))
        s_ps = ctx.enter_context(nc.psum_tensor("s_ps", [1, NJ], f32))

        offs_v = mini_sb[:, :1].bitcast(i32)         # [128,1] gather offsets
        mskS_v = mini_sb[:R, 1 : 1 + NJ]             # [64,60] suffix mask
        mskC_v = mini_sb[:, 1 + NJ : 2 + NJ]         # [128,1] correct mask
        zb64_v = mini_sb[:R, 2 + NJ : 3 + NJ]        # [64,1] zeros (act bias)
        zb1_v = mini_sb[:1, 2 + NJ : 3 + NJ]         # [1,1] zeros (act bias)

        @blk.sync
        def _(sync):
            sync.dma_start(mini_sb[:], mini).then_inc(s_mini, 16)
            sync.wait_ge(s_act, 3)
            sync.dma_start(out, res_sb[:]).then_inc(s_out, 16)

        @blk.scalar
        def _(scalar):
            scalar.dma_start(wt_sb[:], wtst).then_inc(s_wt, 16)
            scalar.dma_start(pair_sb[:], pairm).then_inc(s_pm, 16)
            # warm the activation tables while the DMAs are in flight
            scalar.wait_ge(s_w, 1)
            scalar.activation(
                out=warm_sb[:, 1:2], in_=warm_sb[:, :1], func=AF.Exp,
                bias=warm_sb[:1, :1],
            )
            scalar.activation(
                out=warm_sb[:, 2:3], in_=warm_sb[:, :1], func=AF.Ln,
                bias=warm_sb[:1, :1],
            )
            scalar.activation(
                out=warm_sb[:, 3:4], in_=warm_sb[:, :1], func=AF.Relu,
                bias=warm_sb[:1, :1],
            )
            scalar.wait_ge(s_pe, 1)
            scalar.activation(
                out=e_sb[:], in_=t_ps[:], func=AF.Exp, bias=zb64_v
            ).then_inc(s_act, 1)
            scalar.wait_ge(s_pe, 3)
            scalar.activation(
                out=lse_sb[:],
                in_=s_ps[:],
                func=AF.Ln,
                bias=zb1_v,
                accum_out=rs_sb[:],
            ).then_inc(s_act, 1)
            scalar.wait_ge(s_act, 2)
            # res = relu(rs - C); every loss term is >= 0 so relu is exact
            scalar.activation(
                out=res_sb[:],
                in_=c_ps[:],
                func=AF.Relu,
                bias=rs_sb[:1, :1],
                scale=-1.0,
            ).then_inc(s_act, 1)

        @blk.gpsimd
        def _(gpsimd):
            gpsimd.wait_ge(s_mini, 16)
            gpsimd.indirect_dma_start(
                out=st_sb[:],
                out_offset=None,
                in_=enc,
                in_offset=bass.IndirectOffsetOnAxis(ap=offs_v[:], axis=0),
            ).then_inc(s_g, 16)

        @blk.vector
        def _(vector):
            vector.memset(warm_sb[:], 1.0).then_inc(s_w, 1)
            vector.wait_ge(s_wt, 16)
            vector.wait_ge(s_g, 16)
            vector.tensor_mul(
                out=prod[:], in0=st_sb[:], in1=wt_sb[:]
            ).then_inc(s_dve, 1)
            vector.wait_ge(s_dve, 1)
            vector.tensor_reduce(
                out=red_sb[:],
                in_=prod[:],
                axis=mybir.AxisListType.X,
                op=mybir.AluOpType.add,
            ).then_inc(s_dve, 1)

        @blk.tensor
        def _(tensor):
            tensor.wait_ge(s_pm, 16)
            tensor.wait_ge(s_dve, 2)
            # t[64,1] = pairM^T @ red  (recombine half-row dots)
            tensor.matmul(
                out=t_ps[:], lhsT=pair_sb[:], rhs=red_sb[:], start=True, stop=True
            ).then_inc(s_pe, 1)
            # C = red . mskC2 (correct-logit sum; halves add transparently)
            tensor.matmul(
                out=c_ps[:], lhsT=red_sb[:], rhs=mskC_v, start=True, stop=True
            ).then_inc(s_pe, 1)
            tensor.wait_ge(s_act, 1)
            tensor.matmul(
                out=s_ps[:], lhsT=e_sb[:, :1], rhs=mskS_v, start=True, stop=True
            ).then_inc(s_pe, 1)

    # trim the end-of-program all-engine barrier (drain + EVSEM butterfly):
    # engines can halt independently; the final drain flushes the out-DMA
    for _b in nc.m.functions[0].blocks:
        if _b.name.endswith("_end"):
            _tail_keep = [
                i
                for i in _b.instructions
                if not (
                    type(i).__name__ == "InstDrain" or i.name.startswith("barrier_")
                )
            ]
            del _b.instructions[:]
            _b.instructions.extend(_tail_keep)
    return nc


def _get_nc():
    if "nc" not in _CACHE:
        _CACHE["nc"] = _build_nc()
    return _CACHE["nc"]


def _build_mini_consts():
    """Constant part of the mini tensor (masks + zeros); col 0 (offsets)
    filled per core."""
    m = np.zeros((V, MCOLS), dtype=np.float32)
    # suffix mask (rows 0..63 only): msk[b*T+k, b2*(T-2)+j] = (b==b2, k>=j+2)
    for b in range(B_LOC):
        for k in range(T):
            for j in range(T - 2):
                if k >= j + 2:
                    m[b * T + k, 1 + b * (T - 2) + j] = 1.0
    # correct mask on all 128 half-rows: k >= 2 (halves sum via the matmul)
    for v in range(V):
        if (v % R) % T >= 2:
            m[v, 1 + NJ] = 1.0
    return m


def _build_pairm():
    p = np.zeros((V, R), dtype=np.float32)
    for v in range(V):
        p[v, v % R] = 1.0
    return p


def kernel(encoder_output, his_turn_end_ids, w_ih, w_hh, b_ih, b_hh, fc_w, fc_b):
    from concourse import bass_utils

    nc = _get_nc()
    enc = np.ascontiguousarray(np.asarray(encoder_output, dtype=np.float32))
    ids = np.asarray(his_turn_end_ids)
    w_t = np.asarray(fc_w, dtype=np.float32)[0, H:]  # [E]

    mini = _build_mini_consts()
    pairm = _build_pairm()
    # w_t halves: partition v holds w_t[512*(v//64) : 512*(v//64)+512]
    wtst = np.empty((V, EH), dtype=np.float32)
    wtst[:R] = w_t[None, :EH]
    wtst[R:] = w_t[None, EH:]
    wtst = np.ascontiguousarray(wtst)

    in_maps = []
    for c in range(NCORES):
        b0 = c * B_LOC
        enc_l = enc[b0 : b0 + B_LOC].reshape(B_LOC * S * 2, EH)
        gidx = (
            ids[b0 : b0 + B_LOC].astype(np.int64)
            + (np.arange(B_LOC, dtype=np.int64) * S)[:, None]
        ).reshape(R)
        # half-row offsets: v<64 -> 2*gidx[v], v>=64 -> 2*gidx[v-64]+1
        offs = np.concatenate([2 * gidx, 2 * gidx + 1]).astype(np.int32)
        m = mini.copy()
        m[:, 0] = offs.view(np.float32)
        in_maps.append(
            {
                "enc": enc_l,
                "mini": np.ascontiguousarray(m),
                "wtst": wtst,
                "pairm": pairm,
            }
        )

    try:
        res = bass_utils.run_bass_kernel_spmd(
            nc, in_maps, core_ids=list(range(NCORES))
        )
    except ModuleNotFoundError:
        # ambient BASS_TRACE with no NTFF hook module on this image --
        # rerun with tracing hard-disabled
        import os

        os.environ["BASS_NEVER_TRACE"] = "1"
        res = bass_utils.run_bass_kernel_spmd(
            nc, in_maps, core_ids=list(range(NCORES))
        )
    _CACHE["last_results"] = res
    total = sum(float(r["out"][0, 0]) for r in res.results)
    return np.float32(total / (B * (T - 2)))
